# revision 10
# baseline (speedup 1.0000x reference)
"""Self-contained Trainium2 Bass kernel for the 2-layer GCN problem.

kernel(src, dst, vals, x, W1, W2) -> [80000, 40] float32 logits,
computed as  A @ (relu((A @ x) @ W1) @ W2)  on 8 NeuronCores.

Strategy: nodes sharded round-robin across cores in 128-node slots
(graph-parallel); W1/W2 replicated; the cross-partition z2 exchange is an
on-device AllGather.  SpMM = banked int16 dma_gather of 256B table rows
(4 SWDGE queues) + per-chunk selection-matrix matmuls accumulated in PSUM.
"""
import numpy as np
import ml_dtypes
import jax
from jax.sharding import Mesh, PartitionSpec, NamedSharding
from jax.experimental.shard_map import shard_map

import concourse.bass as bass
import concourse.bacc as bacc
import concourse.tile as tile
import concourse.mybir as mybir
from concourse import bass2jax
from concourse.bass2jax import _bass_exec_p, install_neuronx_cc_hook, partition_id_tensor
from concourse.masks import make_identity

NUM_NODES = 80000
NUM_EDGES = 1280000


import numpy as np

NC, P, GRP = 8, 128, 1024
SUPER = 8
SLABMAX = 48


def build_layout(src, dst, vals, n_nodes, banksz=32768):
    E = src.shape[0]
    NBLK = -(-n_nodes // GRP)
    TAB = NC * NBLK * P
    NBANK = -(-TAB // banksz)

    n = np.arange(n_nodes)
    c_of = (n // P) % NC
    j_of = n // GRP
    s_of = n % P
    table_row = (c_of * NBLK + j_of) * P + s_of

    pdst = table_row[dst]
    beta = (pdst // banksz).astype(np.int64)
    lidx = (pdst % banksz).astype(np.int16)

    ec, ej, es = c_of[src], j_of[src], s_of[src]

    key = (ec * NBLK + ej) * NBANK + beta
    # secondary sort by dst row for HBM locality within gather chunks
    ord_e = np.lexsort((lidx, key))
    ks = key[ord_e]
    first = np.r_[0, np.flatnonzero(np.diff(ks)) + 1]
    group_start = np.zeros(E, np.int64)
    group_start[first] = first
    group_start = np.maximum.accumulate(group_start)
    k_in_group = np.arange(E) - group_start

    cnt = np.bincount(key, minlength=NC * NBLK * NBANK).reshape(NC, NBLK, NBANK)
    Kc = -(-cnt // P)
    Kg = Kc.max(axis=0)                       # [NBLK, NBANK]
    # ensure every block has at least one chunk (psum must be written)
    empty = Kg.sum(axis=1) == 0
    Kg[empty, 0] = 1

    NSB = -(-NBLK // SUPER)
    chunk_base = np.zeros((NBLK, NBANK), np.int64)
    slabs = []            # (start_chunk, n_chunks, bank, superblock)
    chunk_block = []
    pos = 0
    for S in range(NSB):
        js = range(S * SUPER, min((S + 1) * SUPER, NBLK))
        for b in range(NBANK):
            run_start = pos
            for j in js:
                chunk_base[j, b] = pos
                pos += Kg[j, b]
                chunk_block.extend([j] * Kg[j, b])
            run_len = pos - run_start
            o = run_start
            while run_len > 0:
                take = min(run_len, SLABMAX)
                slabs.append((o, take, b, S))
                o += take
                run_len -= take
    NCHUNK = pos
    chunk_block = np.asarray(chunk_block)

    gidx = np.zeros((NC, NCHUNK * P), np.int16)
    gval = np.zeros((NC, NCHUNK * P), np.float32)
    gsrel = np.zeros((NC, NCHUNK * P), np.float32)
    echunk = chunk_base[ej[ord_e], beta[ord_e]] + k_in_group // P
    epos = echunk * P + (k_in_group % P)
    core_o = ec[ord_e]
    lidx_o = lidx[ord_e]
    val_o = vals[ord_e]
    srel_o = es[ord_e].astype(np.float32)
    for ci in range(NC):
        m = core_o == ci
        gidx[ci, epos[m]] = lidx_o[m]
        gval[ci, epos[m]] = val_o[m]
        gsrel[ci, epos[m]] = srel_o[m]

    blk_first = np.full(NBLK, 1 << 60, np.int64)
    blk_last = np.full(NBLK, -1, np.int64)
    for j in range(NBLK):
        for b in range(NBANK):
            if Kg[j, b] > 0:
                blk_first[j] = min(blk_first[j], chunk_base[j, b])
                blk_last[j] = max(blk_last[j], chunk_base[j, b] + Kg[j, b] - 1)
    # per-superblock first/last chunk (one PSUM accumulation group per bank)
    sb_first = np.zeros(NSB, np.int64)
    sb_last = np.zeros(NSB, np.int64)
    for S in range(NSB):
        js = range(S * SUPER, min((S + 1) * SUPER, NBLK))
        sb_first[S] = min(chunk_base[j, b] for j in js for b in range(NBANK)
                          if Kg[j, b] > 0)
        sb_last[S] = max(chunk_base[j, b] + Kg[j, b] - 1 for j in js
                         for b in range(NBANK) if Kg[j, b] > 0)

    return dict(
        NBLK=NBLK, TAB=TAB, NBANK=NBANK, NCHUNK=NCHUNK, NSB=NSB, banksz=banksz,
        table_row=table_row, Kg=Kg, chunk_base=chunk_base,
        chunk_block=chunk_block, slabs=slabs,
        gidx=gidx, gval=gval, gsrel=gsrel,
        blk_first=blk_first, blk_last=blk_last, sb_first=sb_first, sb_last=sb_last,
    )


def wrap_indices(gidx_core):
    """[NCHUNK*128] int16 -> [128, NCHUNK*8] wrapped+replicated tile."""
    n = gidx_core.shape[0]
    wrap = gidx_core.reshape(n // 16, 16).T          # [16, n/16]
    return np.tile(wrap, (8, 1)).copy()





F_IN = 64      # x features
F_HID = 128
F_OUT = 40
FE = 128       # bf16 elements per 256B table row

bf16 = mybir.dt.bfloat16
f32 = mybir.dt.float32


def build_nc(L, reps=1, debug_outs=False, shared_out=True, ag_pieces=1,
             only_phase=None):
    NBLK, TAB, NBANK, NCHUNK, NSB = (
        L["NBLK"], L["TAB"], L["NBANK"], L["NCHUNK"], L["NSB"])
    banksz = L["banksz"]
    slabs = L["slabs"]
    chunk_block = L["chunk_block"]
    sb_first, sb_last = L["sb_first"], L["sb_last"]
    maxslab = max(s[1] for s in slabs)

    nc = bacc.Bacc("TRN2", target_bir_lowering=False, debug=False, num_devices=NC,
                   num_swdge_queues=4)
    xtab = nc.dram_tensor("xtab", [TAB, FE], bf16, kind="ExternalInput")
    z2in = None
    if only_phase == 3:
        z2in = nc.dram_tensor("z2in", [NBLK * P, FE], bf16, kind="ExternalInput")
    gidx = nc.dram_tensor("gidx", [P, NCHUNK * 8], mybir.dt.int16, kind="ExternalInput")
    gval = nc.dram_tensor("gval", [P, NCHUNK], f32, kind="ExternalInput")
    gsrel = nc.dram_tensor("gsrel", [P, NCHUNK], f32, kind="ExternalInput")
    w1 = nc.dram_tensor("w1", [F_IN, F_HID], bf16, kind="ExternalInput")
    w2 = nc.dram_tensor("w2", [F_HID, 64], bf16, kind="ExternalInput")
    iota_in = nc.dram_tensor("iota", [P, P], bf16, kind="ExternalInput")
    out_ext = nc.dram_tensor("out", [NBLK * P, F_OUT], f32, kind="ExternalOutput")
    if debug_outs:
        z1dbg = nc.dram_tensor("z1dbg", [NBLK * P, F_IN], f32, kind="ExternalOutput")
        z2dbg = nc.dram_tensor("z2dbg", [NBLK * P, 64], f32, kind="ExternalOutput")

    def bank_rows(b):
        return slice(b * banksz, min((b + 1) * banksz, TAB))

    with tile.TileContext(nc) as tc:
        with (
            tc.tile_pool(name="cons", bufs=1) as cons,
            tc.tile_pool(name="sbuf", bufs=3) as sbuf,
            tc.tile_pool(name="sv", bufs=12) as svp,
            tc.tile_pool(name="dense", bufs=2) as dns,
            tc.tile_pool(name="psum", bufs=2, space="PSUM") as psum,
            tc.tile_pool(name="psd", bufs=2, space="PSUM") as psd,
            tc.tile_pool(name="dram", bufs=1, space="DRAM") as dram,
        ):
            iota_t = cons.tile([P, P], bf16)
            ident_t = cons.tile([P, P], bf16)
            make_identity(nc, ident_t[:])
            w1_t = cons.tile([F_IN, F_HID], bf16)
            w2_t = cons.tile([F_HID, 64], bf16)
            idx_t = cons.tile([P, NCHUNK * 8], mybir.dt.int16)
            val_t = cons.tile([P, NCHUNK], f32)
            srel_t = cons.tile([P, NCHUNK], f32)
            nc.sync.dma_start(out=iota_t[:], in_=iota_in[:, :])
            nc.sync.dma_start(out=w1_t[:], in_=w1[:, :])
            nc.sync.dma_start(out=w2_t[:], in_=w2[:, :])
            nc.sync.dma_start(out=idx_t[:], in_=gidx[:, :])
            nc.sync.dma_start(out=val_t[:], in_=gval[:, :])
            nc.sync.dma_start(out=srel_t[:], in_=gsrel[:, :])

            z2loc = dram.tile([NBLK * P, FE], bf16)
            z2tab = dram.tile([TAB, FE], bf16,
                              addr_space=("Shared" if shared_out else "Local"))

            def spmm_phase(table, fcols, on_block_done, gtag,
                           on_super_done=None):
                """Gather+selection-matmul over the chunk stream.
                table(b) -> AP of bank b rows; fcols = #feature cols used.
                on_block_done(j, psum_ap_slice) called when block j complete."""
                for S in range(NSB):
                    jlo = S * SUPER
                    jhi = min((S + 1) * SUPER, NBLK)
                    acc = psum.tile([P, 64 * (jhi - jlo)], f32, tag="acc")
                    for si, (c0, Ln, b, SS) in enumerate(slabs):
                        if SS != S:
                            continue
                        g3 = sbuf.tile([P, maxslab, FE], bf16, tag=gtag)
                        nc.gpsimd.dma_gather(
                            g3[:, 0:Ln, :],
                            table(b),
                            idx_t[:, c0 * 8:(c0 + Ln) * 8],
                            Ln * P,
                            Ln * P,
                            FE,
                            single_packet=False,
                            queue_num=(si % 4),
                        )
                        for t in range(Ln):
                            ch = c0 + t
                            j = int(chunk_block[ch])
                            jj = j - jlo
                            sv = svp.tile([P, P], bf16, tag="sv")
                            nc.vector.tensor_scalar(
                                out=sv[:], in0=iota_t[:],
                                scalar1=srel_t[:, ch:ch + 1],
                                scalar2=val_t[:, ch:ch + 1],
                                op0=mybir.AluOpType.is_equal,
                                op1=mybir.AluOpType.mult,
                            )
                            nc.tensor.matmul(
                                out=acc[:, 64 * jj:64 * jj + fcols],
                                lhsT=sv[:],
                                rhs=g3[:, t, 0:fcols],
                                start=(ch == sb_first[S]),
                                stop=(ch == sb_last[S]),
                                skip_group_check=True,
                            )
                    for j in range(jlo, jhi):
                        jj = j - jlo
                        on_block_done(j, acc[:, 64 * jj:64 * jj + fcols])
                    if on_super_done is not None:
                        on_super_done(S, jlo, jhi)

            # ---- phase 1: z1 = A@x ; dense chain ; z2 shard ----
            def phase1_block(j, acc_ap):
                z1_sb = dns.tile([P, F_IN], bf16, tag="z1")
                nc.vector.tensor_copy(out=z1_sb[:], in_=acc_ap)
                if debug_outs:
                    z1f = dns.tile([P, F_IN], f32, tag="z1f")
                    nc.vector.tensor_copy(out=z1f[:], in_=acc_ap)
                    nc.sync.dma_start(out=z1dbg[j * P:(j + 1) * P, :], in_=z1f[:])
                pt = psd.tile([F_IN, P], bf16, tag="pt")
                nc.tensor.transpose(out=pt[:], in_=z1_sb[:], identity=ident_t[:])
                z1t = dns.tile([F_IN, P], bf16, tag="z1t")
                nc.vector.tensor_copy(out=z1t[:], in_=pt[:])
                ph = psd.tile([F_HID, P], f32, tag="pd")
                nc.tensor.matmul(out=ph[:], lhsT=w1_t[:], rhs=z1t[:],
                                 start=True, stop=True)
                ht = dns.tile([F_HID, P], bf16, tag="ht")
                nc.vector.tensor_scalar_max(out=ht[:], in0=ph[:], scalar1=0.0)
                pz = psd.tile([P, 64], f32, tag="pd")
                nc.tensor.matmul(out=pz[:], lhsT=ht[:], rhs=w2_t[:],
                                 start=True, stop=True)
                z2_sb = dns.tile([P, 64], bf16, tag="z2")
                nc.vector.tensor_copy(out=z2_sb[:], in_=pz[:])
                if debug_outs:
                    z2f = dns.tile([P, 64], f32, tag="z2f")
                    nc.vector.tensor_copy(out=z2f[:], in_=pz[:])
                    nc.sync.dma_start(out=z2dbg[j * P:(j + 1) * P, :], in_=z2f[:])
                nc.sync.dma_start(
                    out=z2loc[j * P:(j + 1) * P, 0:64], in_=z2_sb[:])
                if only_phase == 1:
                    o1 = dns.tile([P, 64], f32, tag="o1")
                    nc.vector.tensor_copy(out=o1[:], in_=pz[:])
                    nc.sync.dma_start(
                        out=out_ext[j * P:(j + 1) * P, :], in_=o1[:, 0:F_OUT])

            # ---- phase 2 (interleaved): piecewise allgather of z2 shards ----
            npieces = max(1, min(ag_pieces, NSB))

            sb_per_piece = -(-NSB // npieces)

            def ag_piece(S, jlo, jhi):
                last_of_piece = (S + 1) % sb_per_piece == 0 or S == NSB - 1
                if not last_of_piece:
                    return
                p0 = (S // sb_per_piece) * sb_per_piece
                rlo = p0 * SUPER * P
                rhi = jhi * P
                out3 = z2tab[:].rearrange("(c r) f -> c r f", c=NC)
                nc.gpsimd.collective_compute(
                    "AllGather",
                    mybir.AluOpType.bypass,
                    replica_groups=[list(range(NC))],
                    ins=[z2loc[rlo:rhi, :].opt()],
                    outs=[out3[:, rlo:rhi, :].opt()],
                )

            if only_phase == 3:
                nc.sync.dma_start(out=z2loc[:, :], in_=z2in[:, :])
                out3 = z2tab[:].rearrange("(c r) f -> c r f", c=NC)
                nc.gpsimd.collective_compute(
                    "AllGather", mybir.AluOpType.bypass,
                    replica_groups=[list(range(NC))],
                    ins=[z2loc[:, :].opt()],
                    outs=[out3[:, :, :].opt()],
                )
            else:
                spmm_phase(lambda b: xtab[bank_rows(b), :], F_IN, phase1_block,
                           "g1", on_super_done=ag_piece)

            # ---- phase 3: out = A@z2 ----
            def phase3_block(j, acc_ap):
                o_sb = dns.tile([P, F_OUT], f32, tag="o")
                nc.vector.tensor_copy(out=o_sb[:], in_=acc_ap)
                nc.sync.dma_start(
                    out=out_ext[j * P:(j + 1) * P, :], in_=o_sb[:])

            if only_phase != 1:
                spmm_phase(lambda b: z2tab[bank_rows(b), :], F_OUT,
                           phase3_block, "g2")

    nc.compile()
    return nc


def pack_inputs(L, x, W1, W2):
    """Returns per-core in_maps list."""
    TAB, NCHUNK = L["TAB"], L["NCHUNK"]
    xtab = np.zeros((TAB, FE), ml_dtypes.bfloat16)
    xtab[L["table_row"], 0:F_IN] = x.astype(ml_dtypes.bfloat16)
    w1b = W1.astype(ml_dtypes.bfloat16)                       # [64, 128]
    w2b = np.zeros((F_HID, 64), ml_dtypes.bfloat16)
    w2b[:, 0:F_OUT] = W2.astype(ml_dtypes.bfloat16)
    iota = np.tile(np.arange(P, dtype=np.float32), (P, 1)).astype(ml_dtypes.bfloat16)

    in_maps = []
    for c in range(NC):
        n = NCHUNK * P
        wrap = L["gidx"][c].reshape(n // 16, 16).T            # [16, n/16]
        idx_tile = np.tile(wrap, (8, 1)).copy()
        val_t = L["gval"][c].reshape(NCHUNK, P).T.copy()      # [P, NCHUNK]
        srel_t = L["gsrel"][c].reshape(NCHUNK, P).T.copy()
        in_maps.append({
            "xtab": xtab, "gidx": idx_tile, "gval": val_t, "gsrel": srel_t,
            "w1": w1b, "w2": w2b, "iota": iota,
        })
    return in_maps


def unpack_output(L, results):
    """results: list of per-core dicts with 'out' [NBLK*128, 40]."""
    outcat = np.concatenate([r["out"] for r in results], axis=0)  # [TAB, 40]
    return outcat[L["table_row"]]


import numpy as np
import jax
from jax.sharding import Mesh, PartitionSpec, NamedSharding
from jax.experimental.shard_map import shard_map

import concourse.mybir as mybir
from concourse import bass2jax
from concourse.bass2jax import _bass_exec_p, install_neuronx_cc_hook, partition_id_tensor


def make_runner(nc, n_cores=8, donate=False):
    install_neuronx_cc_hook()
    partition_name = nc.partition_id_tensor.name if nc.partition_id_tensor else None

    in_names, out_names, out_avals, zero_outs = [], [], [], []
    for alloc in nc.m.functions[0].allocations:
        if not isinstance(alloc, mybir.MemoryLocationSet):
            continue
        name = alloc.memorylocations[0].name
        if alloc.kind == "ExternalInput":
            if name != partition_name:
                in_names.append(name)
        elif alloc.kind == "ExternalOutput":
            out_names.append(name)
            shape = tuple(alloc.tensor_shape)
            dtype = mybir.dt.np(alloc.dtype)
            out_avals.append(jax.core.ShapedArray(shape, dtype))
            zero_outs.append(np.zeros(shape, dtype))
    n_params = len(in_names)
    n_outs = len(out_avals)
    all_in_names = list(in_names) + list(out_names)
    if partition_name is not None:
        all_in_names.append(partition_name)

    def _body(*args):
        operands = list(args)
        if partition_name is not None:
            operands.append(partition_id_tensor())
        outs = _bass_exec_p.bind(
            *operands,
            out_avals=tuple(out_avals),
            in_names=tuple(all_in_names),
            out_names=tuple(out_names),
            lowering_input_output_aliases=(),
            sim_require_finite=True,
            sim_require_nnan=True,
            nc=nc,
        )
        return tuple(outs)

    devices = jax.devices()[:n_cores]
    mesh = Mesh(np.asarray(devices), ("core",))
    in_specs = (PartitionSpec("core"),) * (n_params + n_outs)
    out_specs = (PartitionSpec("core"),) * n_outs
    jit_kwargs = {"keep_unused": True}
    if donate:
        jit_kwargs["donate_argnums"] = tuple(range(n_params, n_params + n_outs))
    fn = jax.jit(
        shard_map(_body, mesh=mesh, in_specs=in_specs, out_specs=out_specs,
                  check_rep=False),
        **jit_kwargs,
    )
    sharding = NamedSharding(mesh, PartitionSpec("core"))

    class Runner:
        def __init__(self):
            self.fn = fn
            self.in_names = in_names
            self.out_names = out_names
            self.n_cores = n_cores
            self.sharding = sharding
            self.zero_outs = zero_outs

        def put_inputs(self, in_maps):
            """in_maps: list of per-core dicts name->np array. Returns list of
            device-resident global arrays (concat along axis 0)."""
            args = []
            for name in in_names:
                cat = np.concatenate([np.asarray(m[name]) for m in in_maps], axis=0)
                args.append(jax.device_put(cat, sharding))
            for z in zero_outs:
                cat = np.concatenate([z] * n_cores, axis=0)
                args.append(jax.device_put(cat, sharding))
            return args

        def __call__(self, args):
            return self.fn(*args)

        def run(self, in_maps):
            """One-shot convenience: returns list of per-core dicts."""
            args = self.put_inputs(in_maps)
            outs = self.fn(*args)
            jax.block_until_ready(outs)
            res = []
            for c in range(n_cores):
                d = {}
                for i, name in enumerate(out_names):
                    arr = np.asarray(outs[i])
                    per = arr.shape[0] // n_cores
                    d[name] = arr[c * per:(c + 1) * per]
                res.append(d)
            return res

    return Runner()


_CACHE = {}


def kernel(src, dst, vals, x, W1, W2):
    src = np.asarray(src); dst = np.asarray(dst)
    vals = np.asarray(vals, dtype=np.float32)
    x = np.asarray(x, dtype=np.float32)
    W1 = np.asarray(W1, dtype=np.float32)
    W2 = np.asarray(W2, dtype=np.float32)

    L = build_layout(src.astype(np.int64), dst.astype(np.int64), vals, NUM_NODES)
    key = "r"
    if key not in _CACHE:
        nc = build_nc(L)
        _CACHE[key] = make_runner(nc)
    r = _CACHE[key]
    in_maps = pack_inputs(L, x, W1, W2)
    results = r.run(in_maps)
    return unpack_output(L, results).astype(np.float32)



# revision 13
# speedup vs baseline: 1.0142x; 1.0142x over previous
"""Self-contained Trainium2 Bass kernel for the 2-layer GCN problem.

kernel(src, dst, vals, x, W1, W2) -> [80000, 40] float32 logits,
computed as  A @ (relu((A @ x) @ W1) @ W2)  on 8 NeuronCores.

Strategy: nodes sharded round-robin across cores in 128-node slots
(graph-parallel); W1/W2 replicated.  SpMM = banked int16 dma_gather of
256B table rows (4 SWDGE queues) + per-chunk selection-matrix matmuls
accumulated in PSUM.  The cross-partition z2 exchange is 4 piecewise
AllGathers into Shared DRAM tensors, issued as phase-1 superblocks
complete so phase-3 gathers overlap the tail of phase 1.
"""
import numpy as np
import ml_dtypes
import jax
from jax.sharding import Mesh, PartitionSpec, NamedSharding
from jax.experimental.shard_map import shard_map

import concourse.bass as bass
import concourse.bacc as bacc
import concourse.tile as tile
import concourse.mybir as mybir
from concourse import bass2jax
from concourse.bass2jax import _bass_exec_p, install_neuronx_cc_hook, partition_id_tensor
from concourse.masks import make_identity

NUM_NODES = 80000
NUM_EDGES = 1280000

NC, P, GRP = 8, 128, 1024
SUPER = 8
SLABMAX = 48
NPIECE = 4

F_IN = 64      # x features
F_HID = 128
F_OUT = 40
FE = 128       # bf16 elements per 256B table row

bf16 = mybir.dt.bfloat16
f32 = mybir.dt.float32


def build_stream(ec, ej, bank, lidx, vals, srel, NBLK, NB, order,
                 guard="block"):
    """Pack an edge list into the banked chunk stream.

    ec/ej: src core/block per edge; bank/lidx: gather bank + row-in-bank
    per edge; order: "Sb" (superblock outer, bank inner — phase 1) or
    "bS" (bank/piece outer — phase 3).  Returns chunk stream dict.
    """
    E = ec.shape[0]
    NSB = -(-NBLK // SUPER)
    lidx = lidx.astype(np.int16)

    key = (ec * NBLK + ej) * NB + bank
    ord_e = np.lexsort((lidx, key))
    ks = key[ord_e]
    first = np.r_[0, np.flatnonzero(np.diff(ks)) + 1]
    group_start = np.zeros(E, np.int64)
    group_start[first] = first
    group_start = np.maximum.accumulate(group_start)
    k_in_group = np.arange(E) - group_start

    cnt = np.bincount(key, minlength=NC * NBLK * NB).reshape(NC, NBLK, NB)
    Kc = -(-cnt // P)
    Kg = Kc.max(axis=0)                       # [NBLK, NB]
    if guard == "block":
        empty = Kg.sum(axis=1) == 0
        Kg[empty, 0] = 1
    else:                                     # per-cell guard (phase 3)
        Kg = np.maximum(Kg, 1)

    chunk_base = np.zeros((NBLK, NB), np.int64)
    slabs = []            # (start_chunk, n_chunks, bank, group_id)
    chunk_block = []
    gb_first = {}
    gb_last = {}
    pos = 0

    if order == "Sb":
        for S in range(NSB):
            js = list(range(S * SUPER, min((S + 1) * SUPER, NBLK)))
            gid = S
            for b in range(NB):
                run_start = pos
                for j in js:
                    chunk_base[j, b] = pos
                    pos += Kg[j, b]
                    chunk_block.extend([j] * Kg[j, b])
                run_len = pos - run_start
                o = run_start
                while run_len > 0:
                    take = min(run_len, SLABMAX)
                    slabs.append((o, take, b, gid))
                    o += take
                    run_len -= take
            gb_first[gid] = min(chunk_base[j, b] for j in js for b in range(NB)
                                if Kg[j, b] > 0)
            gb_last[gid] = max(chunk_base[j, b] + Kg[j, b] - 1 for j in js
                               for b in range(NB) if Kg[j, b] > 0)
    else:  # "bS": bank (piece) outer, superblock inner
        for b in range(NB):
            for S in range(NSB):
                js = list(range(S * SUPER, min((S + 1) * SUPER, NBLK)))
                gid = b * NSB + S
                run_start = pos
                for j in js:
                    chunk_base[j, b] = pos
                    pos += Kg[j, b]
                    chunk_block.extend([j] * Kg[j, b])
                run_len = pos - run_start
                o = run_start
                while run_len > 0:
                    take = min(run_len, SLABMAX)
                    slabs.append((o, take, b, gid))
                    o += take
                    run_len -= take
                gb_first[gid] = run_start
                gb_last[gid] = pos - 1
    NCHUNK = pos
    chunk_block = np.asarray(chunk_block)

    gidx = np.zeros((NC, NCHUNK * P), np.int16)
    gval = np.zeros((NC, NCHUNK * P), np.float32)
    gsrel = np.zeros((NC, NCHUNK * P), np.float32)
    echunk = chunk_base[ej[ord_e], bank[ord_e]] + k_in_group // P
    epos = echunk * P + (k_in_group % P)
    core_o = ec[ord_e]
    gidx[core_o, epos] = lidx[ord_e]
    gval[core_o, epos] = vals[ord_e]
    gsrel[core_o, epos] = srel[ord_e]

    return dict(NCHUNK=NCHUNK, slabs=slabs, chunk_block=chunk_block,
                gb_first=gb_first, gb_last=gb_last, Kg=Kg,
                gidx=gidx, gval=gval, gsrel=gsrel)


def build_layout(src, dst, vals, n_nodes, banksz=32768):
    NBLK = -(-n_nodes // GRP)
    TAB = NC * NBLK * P
    NB1 = -(-TAB // banksz)
    NSB = -(-NBLK // SUPER)

    n = np.arange(n_nodes)
    c_of = (n // P) % NC
    j_of = n // GRP
    s_of = n % P
    table_row = (c_of * NBLK + j_of) * P + s_of

    ec, ej, es = c_of[src], j_of[src], s_of[src]
    srel = es.astype(np.float32)

    # phase 1: dst -> xtab row / bank
    r1 = table_row[dst]
    b1 = r1 // banksz
    l1 = r1 % banksz
    st1 = build_stream(ec, ej, b1, l1, vals, srel, NBLK, NB1, "Sb",
                       guard="block")

    # phase 3: dst -> piece (block range of its owner), piece-local row
    pb = -(-NBLK // NPIECE)
    plens = [min((p + 1) * pb, NBLK) - p * pb for p in range(NPIECE)]
    pjd = j_of[dst]
    pcd = c_of[dst]
    psd_ = s_of[dst]
    p3 = pjd // pb
    l3 = (pcd * np.asarray(plens)[p3] + (pjd - p3 * pb)) * P + psd_
    st3 = build_stream(ec, ej, p3, l3, vals, srel, NBLK, NPIECE, "bS",
                       guard="cell")

    return dict(NBLK=NBLK, TAB=TAB, NB1=NB1, NSB=NSB, banksz=banksz,
                pb=pb, plens=plens, table_row=table_row, st1=st1, st3=st3)


def wrap_cols(a, NCHUNK):
    """[NC, NCHUNK*128] -> per-core [128, NCHUNK*8] wrapped int16 tiles."""
    out = []
    for c in range(NC):
        n = a.shape[1]
        w = a[c].reshape(n // 16, 16).T
        out.append(np.tile(w, (8, 1)).copy())
    return out


def build_nc(L, shared_out=True, only_phase=None):
    NBLK, TAB, NB1, NSB = L["NBLK"], L["TAB"], L["NB1"], L["NSB"]
    banksz = L["banksz"]
    pb, plens = L["pb"], L["plens"]
    st1, st3 = L["st1"], L["st3"]
    NCH1, NCH3 = st1["NCHUNK"], st3["NCHUNK"]
    maxslab = max(s[1] for s in st1["slabs"] + st3["slabs"])

    nc = bacc.Bacc("TRN2", target_bir_lowering=False, debug=False,
                   num_devices=NC, num_swdge_queues=4)
    xtab = nc.dram_tensor("xtab", [TAB, FE], bf16, kind="ExternalInput")
    z2in = None
    if only_phase == 3:
        z2in = nc.dram_tensor("z2in", [NBLK * P, FE], bf16, kind="ExternalInput")
    gidx1 = nc.dram_tensor("gidx1", [P, NCH1 * 8], mybir.dt.int16, kind="ExternalInput")
    gval1 = nc.dram_tensor("gval1", [P, NCH1], f32, kind="ExternalInput")
    gsrel1 = nc.dram_tensor("gsrel1", [P, NCH1], f32, kind="ExternalInput")
    gidx3 = nc.dram_tensor("gidx3", [P, NCH3 * 8], mybir.dt.int16, kind="ExternalInput")
    gval3 = nc.dram_tensor("gval3", [P, NCH3], f32, kind="ExternalInput")
    gsrel3 = nc.dram_tensor("gsrel3", [P, NCH3], f32, kind="ExternalInput")
    w1 = nc.dram_tensor("w1", [F_IN, F_HID], bf16, kind="ExternalInput")
    w2 = nc.dram_tensor("w2", [F_HID, 64], bf16, kind="ExternalInput")
    iota_in = nc.dram_tensor("iota", [P, P], bf16, kind="ExternalInput")
    out_ext = nc.dram_tensor("out", [NBLK * P, F_OUT], f32, kind="ExternalOutput")

    def bank_rows(b):
        return slice(b * banksz, min((b + 1) * banksz, TAB))

    with tile.TileContext(nc) as tc:
        with (
            tc.tile_pool(name="cons", bufs=1) as cons,
            tc.tile_pool(name="sbuf", bufs=3) as sbuf,
            tc.tile_pool(name="sv", bufs=12) as svp,
            tc.tile_pool(name="dense", bufs=2) as dns,
            tc.tile_pool(name="psum", bufs=2, space="PSUM") as psum,
            tc.tile_pool(name="psd", bufs=2, space="PSUM") as psd,
            tc.tile_pool(name="dram", bufs=1, space="DRAM") as dram,
        ):
            iota_t = cons.tile([P, P], bf16)
            ident_t = cons.tile([P, P], bf16)
            make_identity(nc, ident_t[:])
            w1_t = cons.tile([F_IN, F_HID], bf16)
            w2_t = cons.tile([F_HID, 64], bf16)
            idx1_t = cons.tile([P, NCH1 * 8], mybir.dt.int16)
            val1_t = cons.tile([P, NCH1], f32)
            srel1_t = cons.tile([P, NCH1], f32)
            idx3_t = cons.tile([P, NCH3 * 8], mybir.dt.int16)
            val3_t = cons.tile([P, NCH3], f32)
            srel3_t = cons.tile([P, NCH3], f32)
            outacc = cons.tile([P, NBLK * F_OUT], f32)
            nc.sync.dma_start(out=iota_t[:], in_=iota_in[:, :])
            nc.sync.dma_start(out=w1_t[:], in_=w1[:, :])
            nc.sync.dma_start(out=w2_t[:], in_=w2[:, :])
            nc.sync.dma_start(out=idx1_t[:], in_=gidx1[:, :])
            nc.sync.dma_start(out=val1_t[:], in_=gval1[:, :])
            nc.sync.dma_start(out=srel1_t[:], in_=gsrel1[:, :])
            nc.sync.dma_start(out=idx3_t[:], in_=gidx3[:, :])
            nc.sync.dma_start(out=val3_t[:], in_=gval3[:, :])
            nc.sync.dma_start(out=srel3_t[:], in_=gsrel3[:, :])

            z2locp = [dram.tile([plens[p] * P, FE], bf16, name=f"z2locp{p}")
                      for p in range(NPIECE)]
            z2p = [dram.tile([NC * plens[p] * P, FE], bf16, name=f"z2p{p}",
                             addr_space=("Shared" if shared_out else "Local"))
                   for p in range(NPIECE)]

            qctr = [0]

            def do_slabs(slabs_sel, st, idx_t, val_t, srel_t, table_of,
                         fcols, acc_of, gtag):
                """Run gather+selection-matmul for the given slab list."""
                for (c0, Ln, b, gid) in slabs_sel:
                    g3 = sbuf.tile([P, maxslab, FE], bf16, tag=gtag)
                    nc.gpsimd.dma_gather(
                        g3[:, 0:Ln, :],
                        table_of(b),
                        idx_t[:, c0 * 8:(c0 + Ln) * 8],
                        Ln * P,
                        Ln * P,
                        FE,
                        single_packet=False,
                        queue_num=(qctr[0] % 4),
                    )
                    qctr[0] += 1
                    for t in range(Ln):
                        ch = c0 + t
                        j = int(st["chunk_block"][ch])
                        jj = j % SUPER
                        sv = svp.tile([P, P], bf16, tag="sv")
                        nc.vector.tensor_scalar(
                            out=sv[:], in0=iota_t[:],
                            scalar1=srel_t[:, ch:ch + 1],
                            scalar2=val_t[:, ch:ch + 1],
                            op0=mybir.AluOpType.is_equal,
                            op1=mybir.AluOpType.mult,
                        )
                        acc = acc_of(gid)
                        nc.tensor.matmul(
                            out=acc[:, 64 * jj:64 * jj + fcols],
                            lhsT=sv[:],
                            rhs=g3[:, t, 0:fcols],
                            start=(ch == st["gb_first"][gid]),
                            stop=(ch == st["gb_last"][gid]),
                            skip_group_check=True,
                        )

            # ---- phase 1: z1 = A@x ; dense chain ; z2 piece shards ----
            ag_issued = [False] * NPIECE

            def piece_of_block(j):
                return j // pb

            def phase1_block(j, acc_ap):
                z1_sb = dns.tile([P, F_IN], bf16, tag="z1")
                nc.vector.tensor_copy(out=z1_sb[:], in_=acc_ap)
                pt = psd.tile([F_IN, P], bf16, tag="pt")
                nc.tensor.transpose(out=pt[:], in_=z1_sb[:], identity=ident_t[:])
                z1t = dns.tile([F_IN, P], bf16, tag="z1t")
                nc.vector.tensor_copy(out=z1t[:], in_=pt[:])
                ph = psd.tile([F_HID, P], f32, tag="pd")
                nc.tensor.matmul(out=ph[:], lhsT=w1_t[:], rhs=z1t[:],
                                 start=True, stop=True)
                ht = dns.tile([F_HID, P], bf16, tag="ht")
                nc.scalar.activation(out=ht[:], in_=ph[:],
                                     func=mybir.ActivationFunctionType.Relu)
                pz = psd.tile([P, 64], f32, tag="pd")
                nc.tensor.matmul(out=pz[:], lhsT=ht[:], rhs=w2_t[:],
                                 start=True, stop=True)
                z2_sb = dns.tile([P, 64], bf16, tag="z2")
                nc.scalar.copy(out=z2_sb[:], in_=pz[:])
                p = piece_of_block(j)
                jl = j - p * pb
                nc.sync.dma_start(
                    out=z2locp[p][jl * P:(jl + 1) * P, 0:64], in_=z2_sb[:])
                if only_phase == 1:
                    o1 = dns.tile([P, 64], f32, tag="o1")
                    nc.vector.tensor_copy(out=o1[:], in_=pz[:])
                    nc.sync.dma_start(
                        out=out_ext[j * P:(j + 1) * P, :], in_=o1[:, 0:F_OUT])

            def issue_ag(p):
                out3 = z2p[p][:].rearrange("(c r) f -> c r f", c=NC)
                nc.gpsimd.collective_compute(
                    "AllGather",
                    mybir.AluOpType.bypass,
                    replica_groups=[list(range(NC))],
                    ins=[z2locp[p][:].opt()],
                    outs=[out3[:, :, :].opt()],
                )

            if only_phase == 3:
                for p in range(NPIECE):
                    nc.sync.dma_start(
                        out=z2locp[p][:, :],
                        in_=z2in[p * pb * P:(p * pb + plens[p]) * P, :])
                    issue_ag(p)
            else:
                # run phase-1 superblock by superblock
                slabs1 = st1["slabs"]
                acc1 = {}

                for S in range(NSB):
                    jlo, jhi = S * SUPER, min((S + 1) * SUPER, NBLK)
                    acc_t = psum.tile([P, 64 * (jhi - jlo)], f32, tag="acc")
                    acc1[S] = acc_t
                    do_slabs([s for s in slabs1 if s[3] == S], st1,
                             idx1_t, val1_t, srel1_t,
                             lambda b: xtab[bank_rows(b), :], F_IN,
                             lambda gid: acc1[gid], "g1")
                    for j in range(jlo, jhi):
                        jj = j - jlo
                        phase1_block(j, acc_t[:, 64 * jj:64 * jj + F_IN])
                    for p in range(NPIECE):
                        if not ag_issued[p] and jhi >= min((p + 1) * pb, NBLK):
                            ag_issued[p] = True
                            issue_ag(p)

            # ---- phase 3: out = A@z2, piece-major with SBUF accumulation ----
            if only_phase != 1:
                slabs3 = st3["slabs"]
                acc3 = {}
                for pp in range(NPIECE):
                    for S in range(NSB):
                        gid = pp * NSB + S
                        jlo, jhi = S * SUPER, min((S + 1) * SUPER, NBLK)
                        acc_t = psum.tile([P, 64 * (jhi - jlo)], f32, tag="acc")
                        acc3[gid] = acc_t
                        do_slabs([s for s in slabs3 if s[3] == gid], st3,
                                 idx3_t, val3_t, srel3_t,
                                 lambda b: z2p[b][:, :], F_OUT,
                                 lambda g: acc3[g], "g2")
                        for j in range(jlo, jhi):
                            jj = j - jlo
                            src = acc_t[:, 64 * jj:64 * jj + F_OUT]
                            dsts = outacc[:, j * F_OUT:(j + 1) * F_OUT]
                            if pp == 0:
                                nc.scalar.copy(out=dsts, in_=src)
                            else:
                                nc.vector.tensor_tensor(
                                    out=dsts, in0=dsts, in1=src,
                                    op=mybir.AluOpType.add)
                # final output DMA (one per block)
                oview = out_ext[:].rearrange("(j s) f -> s j f", s=P)
                nc.sync.dma_start(
                    out=oview[:, :, :],
                    in_=outacc[:].rearrange("s (j f) -> s j f", f=F_OUT))

    nc.compile()
    return nc


def pack_inputs(L, x, W1, W2):
    """Returns per-core in_maps list."""
    TAB = L["TAB"]
    st1, st3 = L["st1"], L["st3"]
    xtab = np.zeros((TAB, FE), ml_dtypes.bfloat16)
    xtab[L["table_row"], 0:F_IN] = x.astype(ml_dtypes.bfloat16)
    w1b = W1.astype(ml_dtypes.bfloat16)
    w2b = np.zeros((F_HID, 64), ml_dtypes.bfloat16)
    w2b[:, 0:F_OUT] = W2.astype(ml_dtypes.bfloat16)
    iota = np.tile(np.arange(P, dtype=np.float32), (P, 1)).astype(ml_dtypes.bfloat16)

    idx1 = wrap_cols(st1["gidx"], st1["NCHUNK"])
    idx3 = wrap_cols(st3["gidx"], st3["NCHUNK"])

    in_maps = []
    for c in range(NC):
        m = {
            "xtab": xtab,
            "gidx1": idx1[c],
            "gval1": st1["gval"][c].reshape(st1["NCHUNK"], P).T.copy(),
            "gsrel1": st1["gsrel"][c].reshape(st1["NCHUNK"], P).T.copy(),
            "gidx3": idx3[c],
            "gval3": st3["gval"][c].reshape(st3["NCHUNK"], P).T.copy(),
            "gsrel3": st3["gsrel"][c].reshape(st3["NCHUNK"], P).T.copy(),
            "w1": w1b, "w2": w2b, "iota": iota,
        }
        in_maps.append(m)
    return in_maps


def unpack_output(L, results):
    """results: list of per-core dicts with 'out' [NBLK*128, 40]."""
    outcat = np.concatenate([r["out"] for r in results], axis=0)  # [TAB, 40]
    return outcat[L["table_row"]]


def make_runner(nc, n_cores=8, donate=False):
    install_neuronx_cc_hook()
    partition_name = nc.partition_id_tensor.name if nc.partition_id_tensor else None

    in_names, out_names, out_avals, zero_outs = [], [], [], []
    for alloc in nc.m.functions[0].allocations:
        if not isinstance(alloc, mybir.MemoryLocationSet):
            continue
        name = alloc.memorylocations[0].name
        if alloc.kind == "ExternalInput":
            if name != partition_name:
                in_names.append(name)
        elif alloc.kind == "ExternalOutput":
            out_names.append(name)
            shape = tuple(alloc.tensor_shape)
            dtype = mybir.dt.np(alloc.dtype)
            out_avals.append(jax.core.ShapedArray(shape, dtype))
            zero_outs.append(np.zeros(shape, dtype))
    n_params = len(in_names)
    n_outs = len(out_avals)
    all_in_names = list(in_names) + list(out_names)
    if partition_name is not None:
        all_in_names.append(partition_name)

    def _body(*args):
        operands = list(args)
        if partition_name is not None:
            operands.append(partition_id_tensor())
        outs = _bass_exec_p.bind(
            *operands,
            out_avals=tuple(out_avals),
            in_names=tuple(all_in_names),
            out_names=tuple(out_names),
            lowering_input_output_aliases=(),
            sim_require_finite=True,
            sim_require_nnan=True,
            nc=nc,
        )
        return tuple(outs)

    devices = jax.devices()[:n_cores]
    mesh = Mesh(np.asarray(devices), ("core",))
    in_specs = (PartitionSpec("core"),) * (n_params + n_outs)
    out_specs = (PartitionSpec("core"),) * n_outs
    jit_kwargs = {"keep_unused": True}
    if donate:
        jit_kwargs["donate_argnums"] = tuple(range(n_params, n_params + n_outs))
    fn = jax.jit(
        shard_map(_body, mesh=mesh, in_specs=in_specs, out_specs=out_specs,
                  check_rep=False),
        **jit_kwargs,
    )
    sharding = NamedSharding(mesh, PartitionSpec("core"))

    class Runner:
        def __init__(self):
            self.fn = fn
            self.in_names = in_names
            self.out_names = out_names
            self.n_cores = n_cores
            self.sharding = sharding
            self.zero_outs = zero_outs

        def put_inputs(self, in_maps):
            args = []
            for name in in_names:
                cat = np.concatenate([np.asarray(m[name]) for m in in_maps], axis=0)
                args.append(jax.device_put(cat, sharding))
            for z in zero_outs:
                cat = np.concatenate([z] * n_cores, axis=0)
                args.append(jax.device_put(cat, sharding))
            return args

        def __call__(self, args):
            return self.fn(*args)

        def run(self, in_maps):
            args = self.put_inputs(in_maps)
            outs = self.fn(*args)
            jax.block_until_ready(outs)
            res = []
            for c in range(n_cores):
                d = {}
                for i, name in enumerate(out_names):
                    arr = np.asarray(outs[i])
                    per = arr.shape[0] // n_cores
                    d[name] = arr[c * per:(c + 1) * per]
                res.append(d)
            return res

    return Runner()


_CACHE = {}


def kernel(src, dst, vals, x, W1, W2):
    src = np.asarray(src); dst = np.asarray(dst)
    vals = np.asarray(vals, dtype=np.float32)
    x = np.asarray(x, dtype=np.float32)
    W1 = np.asarray(W1, dtype=np.float32)
    W2 = np.asarray(W2, dtype=np.float32)

    L = build_layout(src.astype(np.int64), dst.astype(np.int64), vals, NUM_NODES)
    key = "r"
    if key not in _CACHE:
        nc = build_nc(L)
        _CACHE[key] = make_runner(nc)
    r = _CACHE[key]
    in_maps = pack_inputs(L, x, W1, W2)
    results = r.run(in_maps)
    return unpack_output(L, results).astype(np.float32)


# revision 21
# speedup vs baseline: 1.3119x; 1.2936x over previous
"""Self-contained Trainium2 Bass kernel for the 2-layer GCN problem.

kernel(src, dst, vals, x, W1, W2) -> [80000, 40] float32 logits,
computed as  A @ (relu((A @ x) @ W1) @ W2)  on 8 NeuronCores.

Strategy: nodes sharded round-robin across cores in 128-node slots
(graph-parallel); W1/W2 replicated.  SpMM = banked int16 dma_gather of
256B table rows (4 SWDGE queues) + per-chunk selection-matrix matmuls
accumulated in PSUM.  The cross-partition z2 exchange is 4 piecewise
AllGathers into Shared DRAM tensors, issued as phase-1 superblocks
complete so phase-3 gathers overlap the tail of phase 1.
"""
import numpy as np
import ml_dtypes
import jax
from jax.sharding import Mesh, PartitionSpec, NamedSharding
from jax.experimental.shard_map import shard_map

import concourse.bass as bass
import concourse.bacc as bacc
import concourse.tile as tile
import concourse.mybir as mybir
from concourse import bass2jax
from concourse.bass2jax import _bass_exec_p, install_neuronx_cc_hook, partition_id_tensor
from concourse.masks import make_identity

NUM_NODES = 80000
NUM_EDGES = 1280000

NC, P, GRP = 8, 128, 1024
SUPER = 8
SLABMAX = 48
NPIECE = 4

F_IN = 64      # x features
F_HID = 128
F_OUT = 40
FE = 128       # bf16 elements per 256B table row

bf16 = mybir.dt.bfloat16
f32 = mybir.dt.float32


def build_stream(ec, ej, bank, lidx, vals, srel, NBLK, NB, order,
                 guard="block"):
    """Pack an edge list into the banked chunk stream.

    ec/ej: src core/block per edge; bank/lidx: gather bank + row-in-bank
    per edge; order: "Sb" (superblock outer, bank inner — phase 1) or
    "bS" (bank/piece outer — phase 3).  Returns chunk stream dict.
    """
    E = ec.shape[0]
    NSB = -(-NBLK // SUPER)
    lidx = lidx.astype(np.int16)

    key = (ec * NBLK + ej) * NB + bank
    ord_e = np.lexsort((lidx, key))
    ks = key[ord_e]
    first = np.r_[0, np.flatnonzero(np.diff(ks)) + 1]
    group_start = np.zeros(E, np.int64)
    group_start[first] = first
    group_start = np.maximum.accumulate(group_start)
    k_in_group = np.arange(E) - group_start

    cnt = np.bincount(key, minlength=NC * NBLK * NB).reshape(NC, NBLK, NB)
    Kc = -(-cnt // P)
    Kg = Kc.max(axis=0)                       # [NBLK, NB]
    if guard == "block":
        empty = Kg.sum(axis=1) == 0
        Kg[empty, 0] = 1
    else:                                     # per-cell guard (phase 3)
        Kg = np.maximum(Kg, 1)

    chunk_base = np.zeros((NBLK, NB), np.int64)
    slabs = []            # (start_chunk, n_chunks, bank, group_id)
    chunk_block = []
    gb_first = {}
    gb_last = {}
    pos = 0

    if order == "Sb":
        for S in range(NSB):
            js = list(range(S * SUPER, min((S + 1) * SUPER, NBLK)))
            gid = S
            for b in range(NB):
                run_start = pos
                for j in js:
                    chunk_base[j, b] = pos
                    pos += Kg[j, b]
                    chunk_block.extend([j] * Kg[j, b])
                run_len = pos - run_start
                o = run_start
                while run_len > 0:
                    take = min(run_len, SLABMAX)
                    slabs.append((o, take, b, gid))
                    o += take
                    run_len -= take
            gb_first[gid] = min(chunk_base[j, b] for j in js for b in range(NB)
                                if Kg[j, b] > 0)
            gb_last[gid] = max(chunk_base[j, b] + Kg[j, b] - 1 for j in js
                               for b in range(NB) if Kg[j, b] > 0)
    else:  # "bS": bank (piece) outer, superblock inner
        for b in range(NB):
            for S in range(NSB):
                js = list(range(S * SUPER, min((S + 1) * SUPER, NBLK)))
                gid = b * NSB + S
                run_start = pos
                for j in js:
                    chunk_base[j, b] = pos
                    pos += Kg[j, b]
                    chunk_block.extend([j] * Kg[j, b])
                run_len = pos - run_start
                o = run_start
                while run_len > 0:
                    take = min(run_len, SLABMAX)
                    slabs.append((o, take, b, gid))
                    o += take
                    run_len -= take
                gb_first[gid] = run_start
                gb_last[gid] = pos - 1
    NCHUNK = pos
    chunk_block = np.asarray(chunk_block)

    gidx = np.zeros((NC, NCHUNK * P), np.int16)
    gval = np.zeros((NC, NCHUNK * P), np.float32)
    gsrel = np.zeros((NC, NCHUNK * P), np.float32)
    echunk = chunk_base[ej[ord_e], bank[ord_e]] + k_in_group // P
    epos = echunk * P + (k_in_group % P)
    core_o = ec[ord_e]
    gidx[core_o, epos] = lidx[ord_e]
    gval[core_o, epos] = vals[ord_e]
    gsrel[core_o, epos] = srel[ord_e]

    return dict(NCHUNK=NCHUNK, slabs=slabs, chunk_block=chunk_block,
                gb_first=gb_first, gb_last=gb_last, Kg=Kg,
                gidx=gidx, gval=gval, gsrel=gsrel)


def build_layout(src, dst, vals, n_nodes, banksz=32768):
    NBLK = -(-n_nodes // GRP)
    TAB = NC * NBLK * P
    NB1 = -(-TAB // banksz)
    NSB = -(-NBLK // SUPER)

    n = np.arange(n_nodes)
    c_of = (n // P) % NC
    j_of = n // GRP
    s_of = n % P
    table_row = (c_of * NBLK + j_of) * P + s_of

    ec, ej, es = c_of[src], j_of[src], s_of[src]
    srel = es.astype(np.float32)

    # phase 1: dst -> xtab row / bank
    r1 = table_row[dst]
    b1 = r1 // banksz
    l1 = r1 % banksz
    st1 = build_stream(ec, ej, b1, l1, vals, srel, NBLK, NB1, "Sb",
                       guard="block")

    # phase 3: dst -> piece (block range of its owner), piece-local row
    pb = -(-NBLK // NPIECE)
    plens = [min((p + 1) * pb, NBLK) - p * pb for p in range(NPIECE)]
    pjd = j_of[dst]
    pcd = c_of[dst]
    psd_ = s_of[dst]
    p3 = pjd // pb
    l3 = (pcd * np.asarray(plens)[p3] + (pjd - p3 * pb)) * P + psd_
    st3 = build_stream(ec, ej, p3, l3, vals, srel, NBLK, NPIECE, "bS",
                       guard="cell")

    return dict(NBLK=NBLK, TAB=TAB, NB1=NB1, NSB=NSB, banksz=banksz,
                pb=pb, plens=plens, table_row=table_row, st1=st1, st3=st3)


def wrap_cols(a, NCHUNK):
    """[NC, NCHUNK*128] -> per-core [128, NCHUNK*8] wrapped int16 tiles."""
    out = []
    for c in range(NC):
        n = a.shape[1]
        w = a[c].reshape(n // 16, 16).T
        out.append(np.tile(w, (8, 1)).copy())
    return out


def build_nc(L, shared_out=True, only_phase=None, ablate=None):
    NBLK, TAB, NB1, NSB = L["NBLK"], L["TAB"], L["NB1"], L["NSB"]
    banksz = L["banksz"]
    pb, plens = L["pb"], L["plens"]
    st1, st3 = L["st1"], L["st3"]
    NCH1, NCH3 = st1["NCHUNK"], st3["NCHUNK"]
    maxslab = max(s[1] for s in st1["slabs"] + st3["slabs"])

    nc = bacc.Bacc("TRN2", target_bir_lowering=False, debug=False,
                   num_devices=NC, num_swdge_queues=4)
    xtab = nc.dram_tensor("xtab", [TAB, FE], bf16, kind="ExternalInput")
    z2in = None
    if only_phase == 3:
        z2in = nc.dram_tensor("z2in", [NBLK * P, FE], bf16, kind="ExternalInput")
    gidx1 = nc.dram_tensor("gidx1", [P, NCH1 * 8], mybir.dt.int16, kind="ExternalInput")
    gval1 = nc.dram_tensor("gval1", [P, NCH1], f32, kind="ExternalInput")
    gsrel1 = nc.dram_tensor("gsrel1", [P, NCH1], f32, kind="ExternalInput")
    gidx3 = nc.dram_tensor("gidx3", [P, NCH3 * 8], mybir.dt.int16, kind="ExternalInput")
    gval3 = nc.dram_tensor("gval3", [P, NCH3], f32, kind="ExternalInput")
    gsrel3 = nc.dram_tensor("gsrel3", [P, NCH3], f32, kind="ExternalInput")
    w1 = nc.dram_tensor("w1", [F_IN, F_HID], bf16, kind="ExternalInput")
    w2 = nc.dram_tensor("w2", [F_HID, 64], bf16, kind="ExternalInput")
    iota_in = nc.dram_tensor("iota", [P, P], bf16, kind="ExternalInput")
    out_ext = nc.dram_tensor("out", [NBLK * P, F_OUT], f32, kind="ExternalOutput")

    def bank_rows(b):
        return slice(b * banksz, min((b + 1) * banksz, TAB))

    with tile.TileContext(nc) as tc:
        with (
            tc.tile_pool(name="cons", bufs=1) as cons,
            tc.tile_pool(name="sbuf", bufs=4) as sbuf,
            tc.tile_pool(name="sv", bufs=32) as svp,
            tc.tile_pool(name="dense", bufs=2) as dns,
            tc.tile_pool(name="psum", bufs=2, space="PSUM") as psum,
            tc.tile_pool(name="psd", bufs=2, space="PSUM") as psd,
            tc.tile_pool(name="dram", bufs=1, space="DRAM") as dram,
        ):
            iota_t = cons.tile([P, P], bf16)
            ident_t = cons.tile([P, P], bf16)
            make_identity(nc, ident_t[:])
            w1_t = cons.tile([F_IN, F_HID], bf16)
            w2_t = cons.tile([F_HID, 64], bf16)
            idx1_t = cons.tile([P, NCH1 * 8], mybir.dt.int16)
            val1_t = cons.tile([P, NCH1], f32)
            srel1_t = cons.tile([P, NCH1], f32)
            idx3_t = cons.tile([P, NCH3 * 8], mybir.dt.int16)
            val3_t = cons.tile([P, NCH3], f32)
            srel3_t = cons.tile([P, NCH3], f32)
            outacc = cons.tile([P, NBLK * F_OUT], f32)
            nc.sync.dma_start(out=iota_t[:], in_=iota_in[:, :])
            nc.sync.dma_start(out=w1_t[:], in_=w1[:, :])
            nc.sync.dma_start(out=w2_t[:], in_=w2[:, :])
            nc.sync.dma_start(out=idx1_t[:], in_=gidx1[:, :])
            nc.sync.dma_start(out=val1_t[:], in_=gval1[:, :])
            nc.sync.dma_start(out=srel1_t[:], in_=gsrel1[:, :])
            nc.sync.dma_start(out=idx3_t[:], in_=gidx3[:, :])
            nc.sync.dma_start(out=val3_t[:], in_=gval3[:, :])
            nc.sync.dma_start(out=srel3_t[:], in_=gsrel3[:, :])

            z2locp = [dram.tile([plens[p] * P, FE], bf16, name=f"z2locp{p}")
                      for p in range(NPIECE)]
            z2p = [dram.tile([NC * plens[p] * P, FE], bf16, name=f"z2p{p}",
                             addr_space=("Shared" if shared_out else "Local"))
                   for p in range(NPIECE)]

            qctr = [0]
            sv_hoist = None
            if ablate == "svhoist":
                sv_hoist = cons.tile([P, P], bf16)
                nc.vector.tensor_scalar(
                    out=sv_hoist[:], in0=iota_t[:],
                    scalar1=srel1_t[:, 0:1], scalar2=val1_t[:, 0:1],
                    op0=mybir.AluOpType.is_equal, op1=mybir.AluOpType.mult)

            def do_slabs(slabs_sel, st, idx_t, val_t, srel_t, table_of,
                         fcols, acc_of, gtag):
                """Run gather+selection-matmul for the given slab list."""
                for (c0, Ln, b, gid) in slabs_sel:
                    g3 = sbuf.tile([P, maxslab, FE], bf16, tag=gtag)
                    nc.gpsimd.dma_gather(
                        g3[:, 0:Ln, :],
                        table_of(b),
                        idx_t[:, c0 * 8:(c0 + Ln) * 8],
                        Ln * P,
                        Ln * P,
                        FE,
                        single_packet=False,
                        queue_num=(qctr[0] % 4),
                    )
                    qctr[0] += 1
                    if ablate == "gonly":
                        continue
                    for t in range(Ln):
                        ch = c0 + t
                        j = int(st["chunk_block"][ch])
                        jj = j % SUPER
                        if ablate == "svhoist":
                            sv = sv_hoist
                        else:
                            sv = svp.tile([P, P], bf16, tag="sv")
                            nc.vector.tensor_scalar(
                                out=sv[:], in0=iota_t[:],
                                scalar1=srel_t[:, ch:ch + 1],
                                scalar2=val_t[:, ch:ch + 1],
                                op0=mybir.AluOpType.is_equal,
                                op1=mybir.AluOpType.mult,
                            )
                        if ablate == "nope":
                            continue
                        acc = acc_of(gid)
                        nc.tensor.matmul(
                            out=acc[:, 64 * jj:64 * jj + fcols],
                            lhsT=sv[:],
                            rhs=g3[:, t, 0:fcols],
                            start=(ch == st["gb_first"][gid]),
                            stop=(ch == st["gb_last"][gid]),
                            skip_group_check=True,
                        )

            # ---- phase 1: z1 = A@x ; dense chain ; z2 piece shards ----
            ag_issued = [False] * NPIECE

            def piece_of_block(j):
                return j // pb

            def phase1_block(j, acc_ap):
                z1_sb = dns.tile([P, F_IN], bf16, tag="z1")
                nc.vector.tensor_copy(out=z1_sb[:], in_=acc_ap)
                pt = psd.tile([F_IN, P], bf16, tag="pt")
                nc.tensor.transpose(out=pt[:], in_=z1_sb[:], identity=ident_t[:])
                z1t = dns.tile([F_IN, P], bf16, tag="z1t")
                nc.vector.tensor_copy(out=z1t[:], in_=pt[:])
                ph = psd.tile([F_HID, P], f32, tag="pd")
                nc.tensor.matmul(out=ph[:], lhsT=w1_t[:], rhs=z1t[:],
                                 start=True, stop=True)
                ht = dns.tile([F_HID, P], bf16, tag="ht")
                nc.scalar.activation(out=ht[:], in_=ph[:],
                                     func=mybir.ActivationFunctionType.Relu)
                pz = psd.tile([P, 64], f32, tag="pd")
                nc.tensor.matmul(out=pz[:], lhsT=ht[:], rhs=w2_t[:],
                                 start=True, stop=True)
                z2_sb = dns.tile([P, 64], bf16, tag="z2")
                nc.scalar.copy(out=z2_sb[:], in_=pz[:])
                p = piece_of_block(j)
                jl = j - p * pb
                nc.sync.dma_start(
                    out=z2locp[p][jl * P:(jl + 1) * P, 0:64], in_=z2_sb[:])
                if only_phase == 1:
                    o1 = dns.tile([P, 64], f32, tag="o1")
                    nc.vector.tensor_copy(out=o1[:], in_=pz[:])
                    nc.sync.dma_start(
                        out=out_ext[j * P:(j + 1) * P, :], in_=o1[:, 0:F_OUT])

            def issue_ag(p):
                out3 = z2p[p][:].rearrange("(c r) f -> c r f", c=NC)
                nc.gpsimd.collective_compute(
                    "AllGather",
                    mybir.AluOpType.bypass,
                    replica_groups=[list(range(NC))],
                    ins=[z2locp[p][:].opt()],
                    outs=[out3[:, :, :].opt()],
                )

            slabs1 = st1["slabs"]
            slabs3 = st3["slabs"]
            acc1 = {}
            acc3 = {}

            def emit_ph1_sb(S):
                jlo, jhi = S * SUPER, min((S + 1) * SUPER, NBLK)
                acc_t = psum.tile([P, 64 * (jhi - jlo)], f32, tag="acc")
                acc1[S] = acc_t
                do_slabs([s for s in slabs1 if s[3] == S], st1,
                         idx1_t, val1_t, srel1_t,
                         lambda b: xtab[bank_rows(b), :], F_IN,
                         lambda gid: acc1[gid], "g1")
                if ablate in ("gonly", "nope"):
                    return
                if ablate in ("nodense", "svhoist"):
                    dr = dns.tile([P, 64 * (jhi - jlo)], bf16, tag="dr")
                    nc.scalar.copy(out=dr[:], in_=acc_t[:])
                    return
                for j in range(jlo, jhi):
                    jj = j - jlo
                    phase1_block(j, acc_t[:, 64 * jj:64 * jj + F_IN])

            def emit_ph3_piece(pp):
                for S in range(NSB):
                    gid = pp * NSB + S
                    jlo, jhi = S * SUPER, min((S + 1) * SUPER, NBLK)
                    acc_t = psum.tile([P, 64 * (jhi - jlo)], f32, tag="acc")
                    acc3[gid] = acc_t
                    do_slabs([s for s in slabs3 if s[3] == gid], st3,
                             idx3_t, val3_t, srel3_t,
                             lambda b: z2p[b][:, :], F_OUT,
                             lambda g: acc3[g], "g2")
                    for j in range(jlo, jhi):
                        jj = j - jlo
                        src = acc_t[:, 64 * jj:64 * jj + F_OUT]
                        dsts = outacc[:, j * F_OUT:(j + 1) * F_OUT]
                        if pp == 0:
                            nc.scalar.copy(out=dsts, in_=src)
                        else:
                            nc.vector.tensor_tensor(
                                out=dsts, in0=dsts, in1=src,
                                op=mybir.AluOpType.add)

            if only_phase == 3:
                for p in range(NPIECE):
                    nc.sync.dma_start(
                        out=z2locp[p][:, :],
                        in_=z2in[p * pb * P:(p * pb + plens[p]) * P, :])
                    issue_ag(p)
                for pp in range(NPIECE):
                    emit_ph3_piece(pp)
            elif only_phase == 1 or ablate is not None:
                for S in range(NSB):
                    emit_ph1_sb(S)
                    if ablate is None:
                        for p in range(NPIECE):
                            if not ag_issued[p] and \
                               min((S + 1) * SUPER, NBLK) >= min((p + 1) * pb, NBLK):
                                ag_issued[p] = True
                                issue_ag(p)
                if ablate is not None:
                    nc.vector.memset(outacc[:, 0:F_OUT], 0.0)
                    nc.sync.dma_start(out=out_ext[0:P, :],
                                      in_=outacc[:, 0:F_OUT])
            else:
                # full: interleave — AGs issued one SB after data-ready;
                # ph3 pieces emitted once their AG has had time to land.
                def ready_sb(p):
                    tgt = min((p + 1) * pb, NBLK)
                    for S in range(NSB):
                        if min((S + 1) * SUPER, NBLK) >= tgt:
                            return S
                    return NSB - 1

                ag_after = {}
                ph3_after = {}
                for p in range(NPIECE):
                    ag_after.setdefault(min(ready_sb(p) + 1, NSB - 1), []).append(p)
                    ph3_after.setdefault(min(ready_sb(p) + 4, NSB - 1), []).append(p)
                for S in range(NSB):
                    emit_ph1_sb(S)
                    for p in ag_after.get(S, []):
                        issue_ag(p)
                    for pp in ph3_after.get(S, []):
                        emit_ph3_piece(pp)

            if only_phase != 1 and ablate is None:
                # final output DMA (one per block)
                oview = out_ext[:].rearrange("(j s) f -> s j f", s=P)
                nc.sync.dma_start(
                    out=oview[:, :, :],
                    in_=outacc[:].rearrange("s (j f) -> s j f", f=F_OUT))

    nc.compile()
    return nc


def pack_inputs(L, x, W1, W2):
    """Returns per-core in_maps list."""
    TAB = L["TAB"]
    st1, st3 = L["st1"], L["st3"]
    xtab = np.zeros((TAB, FE), ml_dtypes.bfloat16)
    xtab[L["table_row"], 0:F_IN] = x.astype(ml_dtypes.bfloat16)
    w1b = W1.astype(ml_dtypes.bfloat16)
    w2b = np.zeros((F_HID, 64), ml_dtypes.bfloat16)
    w2b[:, 0:F_OUT] = W2.astype(ml_dtypes.bfloat16)
    iota = np.tile(np.arange(P, dtype=np.float32), (P, 1)).astype(ml_dtypes.bfloat16)

    idx1 = wrap_cols(st1["gidx"], st1["NCHUNK"])
    idx3 = wrap_cols(st3["gidx"], st3["NCHUNK"])

    in_maps = []
    for c in range(NC):
        m = {
            "xtab": xtab,
            "gidx1": idx1[c],
            "gval1": st1["gval"][c].reshape(st1["NCHUNK"], P).T.copy(),
            "gsrel1": st1["gsrel"][c].reshape(st1["NCHUNK"], P).T.copy(),
            "gidx3": idx3[c],
            "gval3": st3["gval"][c].reshape(st3["NCHUNK"], P).T.copy(),
            "gsrel3": st3["gsrel"][c].reshape(st3["NCHUNK"], P).T.copy(),
            "w1": w1b, "w2": w2b, "iota": iota,
        }
        in_maps.append(m)
    return in_maps


def unpack_output(L, results):
    """results: list of per-core dicts with 'out' [NBLK*128, 40]."""
    outcat = np.concatenate([r["out"] for r in results], axis=0)  # [TAB, 40]
    return outcat[L["table_row"]]


def make_runner(nc, n_cores=8, donate=False):
    install_neuronx_cc_hook()
    partition_name = nc.partition_id_tensor.name if nc.partition_id_tensor else None

    in_names, out_names, out_avals, zero_outs = [], [], [], []
    for alloc in nc.m.functions[0].allocations:
        if not isinstance(alloc, mybir.MemoryLocationSet):
            continue
        name = alloc.memorylocations[0].name
        if alloc.kind == "ExternalInput":
            if name != partition_name:
                in_names.append(name)
        elif alloc.kind == "ExternalOutput":
            out_names.append(name)
            shape = tuple(alloc.tensor_shape)
            dtype = mybir.dt.np(alloc.dtype)
            out_avals.append(jax.core.ShapedArray(shape, dtype))
            zero_outs.append(np.zeros(shape, dtype))
    n_params = len(in_names)
    n_outs = len(out_avals)
    all_in_names = list(in_names) + list(out_names)
    if partition_name is not None:
        all_in_names.append(partition_name)

    def _body(*args):
        operands = list(args)
        if partition_name is not None:
            operands.append(partition_id_tensor())
        outs = _bass_exec_p.bind(
            *operands,
            out_avals=tuple(out_avals),
            in_names=tuple(all_in_names),
            out_names=tuple(out_names),
            lowering_input_output_aliases=(),
            sim_require_finite=True,
            sim_require_nnan=True,
            nc=nc,
        )
        return tuple(outs)

    devices = jax.devices()[:n_cores]
    mesh = Mesh(np.asarray(devices), ("core",))
    in_specs = (PartitionSpec("core"),) * (n_params + n_outs)
    out_specs = (PartitionSpec("core"),) * n_outs
    jit_kwargs = {"keep_unused": True}
    if donate:
        jit_kwargs["donate_argnums"] = tuple(range(n_params, n_params + n_outs))
    fn = jax.jit(
        shard_map(_body, mesh=mesh, in_specs=in_specs, out_specs=out_specs,
                  check_rep=False),
        **jit_kwargs,
    )
    sharding = NamedSharding(mesh, PartitionSpec("core"))

    class Runner:
        def __init__(self):
            self.fn = fn
            self.in_names = in_names
            self.out_names = out_names
            self.n_cores = n_cores
            self.sharding = sharding
            self.zero_outs = zero_outs

        def put_inputs(self, in_maps):
            args = []
            for name in in_names:
                cat = np.concatenate([np.asarray(m[name]) for m in in_maps], axis=0)
                args.append(jax.device_put(cat, sharding))
            for z in zero_outs:
                cat = np.concatenate([z] * n_cores, axis=0)
                args.append(jax.device_put(cat, sharding))
            return args

        def __call__(self, args):
            return self.fn(*args)

        def run(self, in_maps):
            args = self.put_inputs(in_maps)
            outs = self.fn(*args)
            jax.block_until_ready(outs)
            res = []
            for c in range(n_cores):
                d = {}
                for i, name in enumerate(out_names):
                    arr = np.asarray(outs[i])
                    per = arr.shape[0] // n_cores
                    d[name] = arr[c * per:(c + 1) * per]
                res.append(d)
            return res

    return Runner()


_CACHE = {}


def kernel(src, dst, vals, x, W1, W2):
    src = np.asarray(src); dst = np.asarray(dst)
    vals = np.asarray(vals, dtype=np.float32)
    x = np.asarray(x, dtype=np.float32)
    W1 = np.asarray(W1, dtype=np.float32)
    W2 = np.asarray(W2, dtype=np.float32)

    L = build_layout(src.astype(np.int64), dst.astype(np.int64), vals, NUM_NODES)
    key = "r"
    if key not in _CACHE:
        nc = build_nc(L)
        _CACHE[key] = make_runner(nc)
    r = _CACHE[key]
    in_maps = pack_inputs(L, x, W1, W2)
    results = r.run(in_maps)
    return unpack_output(L, results).astype(np.float32)
